# revision 21
# baseline (speedup 1.0000x reference)
# Trainium2 Bass kernel for CrossAttentionFusion — fp8 DoubleRow version.
#
# Reference computation (per batch b):
#   pet_seq = pet_feat[b] viewed as (C, L)^T            L = H*W = 4096, C = 512
#   q = pet_seq @ Wq.T ; k = ct_seq @ Wk.T ; v = ct_seq @ Wv.T   (8 heads, hd=64)
#   x = softmax(q k^T / sqrt(hd)) v                      per head
#   y = LN(pet_seq + x @ Wp.T + bp) * gamma + beta       -> (B, C, H, W)
#
# Sharding: 8 cores = 2 batches x 4 query-row chunks (1024 rows each), no
# collectives (outputs are disjoint (C, 1024) column blocks).
#
# All large matmuls run in fp8(e4m3) with MatmulPerfMode.DoubleRow: operands
# are laid out as [P, 2, *] access patterns and the PE contracts 2*P elements
# per instruction at 0.5 cycles/row — 4x the bf16 MAC throughput.  Scales are
# folded so every fp8 tensor sits near unit std and no extra scaling ops are
# needed:
#   weights are stored *16 (fp8-friendly), q/k copies divide by 16,
#   v copies multiply by 2 (v8 = 32*v), so o = 32*x_num while the softmax
#   denominator (ones column in v8) stays exact; xt8 = o*recip(den) = 32*x
#   lands at unit std; out-proj psum = 16*32*y and the residual add applies
#   1/512; bp is folded into the pet residual host-side.
#
# Score exp is the throughput limit: each [128,1024] PSUM score pair-group is
# turned into fp8 probabilities by EITHER ScalarE (table exp, fp8 out) or DVE
# (Schraudolph: affine to the e4m3 bit pattern, int8 out, bitcast fp8),
# split by a tunable pattern so both engines stay busy.  The [128, 2, 512]
# pair-group layout IS the DoubleRow rhs for the PV matmul (K = 256 keys per
# instruction).
#
# HW gotchas encoded here (CoreSim does not model them):
#  - DVE/ACT ops must have in/out APs at the SAME base partition; any
#    cross-partition move goes through DMA (or a ones-matmul broadcast).
#  - a tensor feeding a float32r matmul must be *written* as float32r.

import numpy as np
import ml_dtypes

import concourse.bacc as bacc
import concourse.bass as bass
import concourse.tile as tile
from concourse import mybir
from concourse import bass_utils
from concourse.alu_op_type import AluOpType
from contextlib import ExitStack

F32 = mybir.dt.float32
F32R = mybir.dt.float32r
F8 = mybir.dt.float8e4
I8 = mybir.dt.int8
DR = mybir.MatmulPerfMode.DoubleRow
E4 = ml_dtypes.float8_e4m3

B, C, H, W = 2, 512, 64, 64
L = H * W                    # 4096
NH, HD = 8, 64
NCORES = 8
LQ = L // 4                  # 1024 query rows per core
NMP = L // 256               # 16 key chunk-pairs
ATT_SCALE = HD ** -0.5       # 1/8
LN_EPS = 1e-5

# Schraudolph exp emitting the e4m3 bit pattern directly as int8:
# bits = trunc(x * 8/ln2 + B8); value = bitcast_e4m3(bits) ~= exp(x).
EXP_A8 = 8.0 / np.log(2.0)
# The Schraudolph bias (and a softmax shift that keeps exp() inside e4m3
# range) is folded into the score matmul: DoubleRow slot 1 of qt8/kt8 is
# all-zero except one element per head (c_q resp. c_k), adding c_q*c_k to
# every raw score.  The DVE exp is then a pure multiply, freeing op1 for a
# max(.,0) clamp (negative int8 bits would bitcast to NaN e4m3).
BIAS_Q, BIAS_K = 5.0, 4.5            # both exactly representable in e4m3
BIAS_PROD = BIAS_Q * BIAS_K          # 22.5 added to every score psum
EXP_SHIFT = (56.04 - BIAS_PROD * EXP_A8 * ATT_SCALE) / EXP_A8   # ~2.043
EXP_BIAS_ACT = -BIAS_PROD * ATT_SCALE - EXP_SHIFT               # ~-4.856

# exp engine split: of each 16 pair-groups, this many go to ScalarE
K_ACT = 7
# phase-1 psum->sbuf copies: which go to DVE instead of ACT
DVE_COPY_K = 8      # of 16 K-proj copies
DVE_COPY_V = 8      # of 16 V-proj copies


def build_nc(debug: bool = False, debug_taps: bool = False, only_first: bool = False):
    nc = bacc.Bacc("TRN2", target_bir_lowering=False, debug=debug,
                   num_devices=NCORES)

    # ---- DRAM I/O ----------------------------------------------------------
    pet8_d = nc.dram_tensor("pet8", [128, 4 * LQ], F8, kind="ExternalInput").ap()
    ctk_d = nc.dram_tensor("ct8k", [128, 4 * L], F8, kind="ExternalInput").ap()
    ctv_d = nc.dram_tensor("ct8v", [128, 4 * L], F8, kind="ExternalInput").ap()
    wq_d = nc.dram_tensor("wq8", [128, 2048], F8, kind="ExternalInput").ap()
    wk_d = nc.dram_tensor("wk8", [128, 2048], F8, kind="ExternalInput").ap()
    wv_d = nc.dram_tensor("wv8", [128, 2048], F8, kind="ExternalInput").ap()
    wp_d = nc.dram_tensor("wp8", [128, 2048], F8, kind="ExternalInput").ap()
    pet_f = nc.dram_tensor("pet32p", [C, LQ], F32, kind="ExternalInput").ap()
    qb_d = nc.dram_tensor("qbias8", [128, LQ], F8, kind="ExternalInput").ap()
    kb_d = nc.dram_tensor("kbias8", [128, L], F8, kind="ExternalInput").ap()
    vp_d = nc.dram_tensor("vpad8", [128, NH * 256], F8, kind="ExternalInput").ap()
    gamma_d = nc.dram_tensor("gamma", [C, 1], F32, kind="ExternalInput").ap()
    beta_d = nc.dram_tensor("beta", [C, 1], F32, kind="ExternalInput").ap()
    out_d = nc.dram_tensor("out", [C, LQ], F32, kind="ExternalOutput").ap()
    taps = {}
    if debug_taps:
        taps["qt"] = nc.dram_tensor("dbg_qt", [128, 4 * 2 * LQ], F8, kind="ExternalOutput").ap()
        taps["kt"] = nc.dram_tensor("dbg_kt", [128, 4 * 2 * L], F8, kind="ExternalOutput").ap()
        taps["vt"] = nc.dram_tensor("dbg_vt", [128, NMP * NH * 256], F8, kind="ExternalOutput").ap()
        taps["xt"] = nc.dram_tensor("dbg_xt", [128, 2 * 2 * LQ], F8, kind="ExternalOutput").ap()
        taps["xres"] = nc.dram_tensor("dbg_xres", [C, LQ], F32, kind="ExternalOutput").ap()
        taps["osb"] = nc.dram_tensor("dbg_osb", [HD + 1, 16 * 512], F32, kind="ExternalOutput").ap()
        taps["s0"] = nc.dram_tensor("dbg_s0", [128, 1024], F32, kind="ExternalOutput").ap()
        taps["p0"] = nc.dram_tensor("dbg_p0", [128, 1024], F8, kind="ExternalOutput").ap()

    NC4 = C // 128

    with tile.TileContext(nc) as tc, ExitStack() as top:
        persist = top.enter_context(tc.tile_pool(name="persist", bufs=1))

        def alloc(shape, dt, tag):
            return persist.tile(shape, dt, tag=tag, name=tag)

        pet32 = [alloc([128, LQ], F32, f"pet32_{i}") for i in range(NC4)]
        wp8 = alloc([128, 2048], F8, "wp8")
        gamma = [alloc([128, 1], F32, f"g_{i}") for i in range(NC4)]
        beta = [alloc([128, 1], F32, f"b_{i}") for i in range(NC4)]

        # Q/K: 4 super-tiles of 2 heads each (64-partition blocks, bases 0/64;
        # the ISA only allows AP base partitions 0/32/64).  Each head's 64
        # dims are stored twice (DoubleRow slots i=0 and i=1, duplicated via
        # SBUF->SBUF DMA), so the score contraction counts every dim twice —
        # the 1/2 is folded into the exp scale.
        qt8 = [alloc([128, 2 * LQ], F8, f"qt8_{t}") for t in range(4)]
        kt8 = [alloc([128, 2 * L], F8, f"kt8_{t}") for t in range(4)]
        # per head+slot: 64 dims, ones column (softmax denominator), then
        # padding to 128 — the only dual-fp8 ldweights row widths walrus
        # accepts are 64/128.  Pad+ones are DMA-preloaded from a host const.
        vt8 = [alloc([128, NH * 256], F8, f"vt8_{mp}") for mp in range(NMP)]
        xt8 = [alloc([128, 2 * LQ], F8, f"xt8_{t}") for t in range(2)]
        xres = [alloc([128, LQ], F32R, f"xr_{i}") for i in range(NC4)]

        # ones used as matmul lhsT for partition reductions / broadcasts
        ones_r = persist.tile([1, 128], F32R, tag="ones_r", name="ones_r")
        ones_c = persist.tile([128, 1], F32R, tag="ones_c", name="ones_c")
        ones_rf = persist.tile([1, 128], F32, tag="ones_rf", name="ones_rf")
        ones_cf = persist.tile([128, 1], F32, tag="ones_cf", name="ones_cf")
        expb = persist.tile([128, 1], F32, tag="expb", name="expb")
        nc.vector.memset(expb[:], EXP_BIAS_ACT)
        nc.vector.memset(ones_rf[:], 1.0)
        nc.vector.memset(ones_cf[:], 1.0)
        nc.vector.tensor_copy(ones_r[:], ones_rf[:])
        nc.vector.tensor_copy(ones_c[:], ones_cf[:])

        # ---- phase 1: projections (fp8 DoubleRow, K=256/instr) -------------
        with tc.tile_pool(name="ph1", bufs=1) as ph1, \
             tc.tile_pool(name="pj", bufs=3, space="PSUM") as pj:
            def p1load(ap_dram, shape, tag):
                t = ph1.tile(shape, F8, tag=tag, name=tag)
                nc.sync.dma_start(t[:], ap_dram)
                return t
            wq8 = p1load(wq_d, [128, 2048], "wq8p")
            pet8 = p1load(pet8_d, [128, 4 * LQ], "pet8")
            wk8 = p1load(wk_d, [128, 2048], "wk8p")
            ct8k = p1load(ctk_d, [128, 4 * L], "ct8k")
            wv8 = p1load(wv_d, [128, 2048], "wv8p")
            ct8v = p1load(ctv_d, [128, 4 * L], "ct8v")
            for i in range(NC4):
                nc.sync.dma_start(pet32[i][:], pet_f[i * 128:(i + 1) * 128, :])
            nc.sync.dma_start(wp8[:], wp_d)
            for i in range(NC4):
                nc.sync.dma_start(gamma[i][:], gamma_d[i * 128:(i + 1) * 128, :])
                nc.sync.dma_start(beta[i][:], beta_d[i * 128:(i + 1) * 128, :])

            def wqk_ap(w, t, g):
                off = (t * 2 + g) * 256
                return w[:, off:off + 256].rearrange("p (i m) -> p i m", i=2)

            def pet_ap(g, lc):
                # [p, 2, 512] contiguous block at g*2048 + lc*1024
                return pet8[:, g * 2048 + lc * 1024:
                            g * 2048 + (lc + 1) * 1024].rearrange(
                    "p (i n) -> p i n", i=2)

            def ctk_ap(g, nc_):
                return ct8k[:, g * 2 * L + nc_ * 1024:
                            g * 2 * L + (nc_ + 1) * 1024].rearrange(
                    "p (i n) -> p i n", i=2)

            def ctv_ap(g, m):
                return ct8v[:, g * 2 * L + m * 256:
                            g * 2 * L + (m + 1) * 256].rearrange(
                    "p (i m) -> p i m", i=2)

            cnt = {"k": 0, "v": 0}

            def p1copy(dst, src, scale, which):
                if which == "q":
                    use_dve = False
                elif which == "k":
                    use_dve = cnt["k"] < DVE_COPY_K; cnt["k"] += 1
                else:
                    use_dve = cnt["v"] < DVE_COPY_V; cnt["v"] += 1
                if use_dve:
                    nc.vector.tensor_scalar(dst, src, float(scale), None,
                                            AluOpType.mult)
                else:
                    nc.scalar.activation(dst, src,
                                         mybir.ActivationFunctionType.Copy,
                                         scale=float(scale))

            # Q: psum group (t) = [128, 1024] covering both lc halves; writes
            # slot 0, then DMA-duplicates into slot 1.
            for t in range(4):
                ps = pj.tile([128, 1024], F32, tag="pj", name="pj")
                for lc in range(2):
                    for g in range(2):
                        nc.tensor.matmul(
                            ps[:, lc * 512:(lc + 1) * 512],
                            wqk_ap(wq8, t, g),
                            pet_ap(g, lc),
                            start=(g == 0), stop=(g == 1), perf_mode=DR)
                # slot 0 of each lc block: free = lc*1024 + i*512 + n
                dst = qt8[t].rearrange("p (c i n) -> p c i n",
                                       i=2, n=512)[:, :, 0, :]
                nc.scalar.activation(
                    dst, ps[:],
                    mybir.ActivationFunctionType.Copy, scale=1.0 / 16)
                nc.sync.dma_start(
                    qt8[t].rearrange("p (c i n) -> p c i n",
                                     i=2, n=512)[:, :, 1, :], qb_d)

            # K: psum group (t,mcp) over token pairs of 512
            for t in range(4):
                for mcp in range(L // 1024):
                    ps = pj.tile([128, 1024], F32, tag="pj", name="pj")
                    for half in range(2):
                        for g in range(2):
                            nc.tensor.matmul(
                                ps[:, half * 512:(half + 1) * 512],
                                wqk_ap(wk8, t, g),
                                ctk_ap(g, 2 * mcp + half),
                                start=(g == 0), stop=(g == 1), perf_mode=DR)
                    # slot 0 of key chunks mcp*8 .. mcp*8+7
                    dst = kt8[t].rearrange("p (c i m) -> p c i m",
                                           i=2, m=128)[:, mcp * 8:(mcp + 1) * 8,
                                                       0, :]
                    p1copy(dst, ps[:], 1.0 / 16, "k")
                nc.sync.dma_start(
                    kt8[t].rearrange("p (c i m) -> p c i m",
                                     i=2, m=128)[:, :, 1, :], kb_d)

            # V: psum group (mp) = [128 tokens, 2x512 (h d)]
            for mp in range(NMP):
                nc.sync.dma_start(vt8[mp][:], vp_d)
                ps = pj.tile([128, 1024], F32, tag="pj", name="pj")
                for half in range(2):
                    m = 2 * mp + half
                    for g in range(2):
                        rhs = wv8[:, g * 1024:(g + 1) * 1024].rearrange(
                            "p (i n) -> p i n", i=2)
                        nc.tensor.matmul(
                            ps[:, half * 512:(half + 1) * 512],
                            ctv_ap(g, m), rhs,
                            start=(g == 0), stop=(g == 1), perf_mode=DR)
                # scatter (half, h, d) -> (h, half, d) blocks of 128
                dst = vt8[mp].rearrange("p (h i d) -> p h i d",
                                        h=NH, i=2)[:, :, :, 0:HD]
                src = ps.rearrange("p (i h d) -> p h i d", i=2, h=NH)
                p1copy(dst, src, 2.0, "v")

        # ---- phase 2: attention + norm + out-proj + LayerNorm --------------
        with tc.tile_pool(name="osb", bufs=1) as osbp, \
             tc.tile_pool(name="ps_s", bufs=1, space="PSUM") as ps_s, \
             tc.tile_pool(name="ps_o", bufs=1, space="PSUM") as ps_o, \
             tc.tile_pool(name="pt", bufs=1) as ptp, \
             tc.tile_pool(name="pp", bufs=2, space="PSUM") as pp, \
             tc.tile_pool(name="nrm", bufs=2) as nrm, \
             tc.tile_pool(name="tmp", bufs=2) as tmp, \
             tc.tile_pool(name="lrows", bufs=1) as lrows, \
             tc.tile_pool(name="yout", bufs=2) as yout:
            stores = {}
            ectr = [0]

            def attention(h, lc):
                t, j = divmod(h, 2)
                pb = 64 * j
                qs = qt8[t][pb:pb + 64, lc * 1024:(lc + 1) * 1024].rearrange(
                    "p (i n) -> p i n", i=2)
                oH = ps_o.tile([128, 512], F32, tag="oH", bufs=2, name="oH")
                for mp in range(NMP):
                    sg = ps_s.tile([128, 1024], F32, tag="sg", bufs=2, name="sg")
                    for half in range(2):
                        m = 2 * mp + half
                        nc.tensor.matmul(
                            sg[:, half * 512:(half + 1) * 512],
                            kt8[t][pb:pb + 64,
                                   m * 256:(m + 1) * 256].rearrange(
                                "p (i m) -> p i m", i=2), qs,
                            start=True, stop=True, perf_mode=DR)
                    use_act = (ectr[0] % 16) < K_ACT
                    ectr[0] += 1
                    if use_act:
                        p8 = ptp.tile([128, 1024], F8, tag="p8a", bufs=2,
                                      name="p8a")
                        nc.scalar.activation(
                            p8[:], sg[:], mybir.ActivationFunctionType.Exp,
                            bias=expb[:], scale=ATT_SCALE)
                        rhs = p8[:]
                    else:
                        p8i = ptp.tile([128, 1024], I8, tag="p8i", bufs=2,
                                       name="p8i")
                        nc.vector.tensor_scalar(
                            p8i[:], sg[:], EXP_A8 * ATT_SCALE, 0.0,
                            AluOpType.mult, AluOpType.max)
                        rhs = p8i[:].bitcast(F8)
                    if debug_taps and h == 0 and lc == 0 and mp == 0:
                        s0 = osbp.tile([128, 1024], F32, tag="dbg_s0",
                                       name="dbg_s0")
                        nc.vector.tensor_copy(s0[:], sg[:])
                        nc.sync.dma_start(taps["s0"], s0[:])
                        p0 = osbp.tile([128, 1024], F8, tag="dbg_p0",
                                       name="dbg_p0")
                        nc.vector.tensor_copy(p0[:], rhs)
                        nc.sync.dma_start(taps["p0"], p0[:])
                    nc.tensor.matmul(
                        oH[:],
                        vt8[mp][:, h * 256:(h + 1) * 256].rearrange(
                            "p (i d) -> p i d", i=2),
                        rhs.rearrange("p (i n) -> p i n", i=2),
                        start=(mp == 0), stop=(mp == NMP - 1), perf_mode=DR)
                o_sb = osbp.tile([HD + 1, 512], F32, tag=f"osb_{h}_{lc}",
                                 name=f"osb_{h}_{lc}")
                if h % 2 == 0:
                    nc.vector.tensor_copy(o_sb[:], oH[0:HD + 1, :])
                else:
                    nc.scalar.copy(o_sb[:], oH[0:HD + 1, :])
                stores[(h, lc)] = o_sb

            def norm_chunk(lc):
                den = osbp.tile([NH, 512], F32, tag=f"den{lc}", name=f"den{lc}")
                for h in range(NH):
                    nc.sync.dma_start(den[h:h + 1, :],
                                      stores[(h, lc)][64:65, :])
                nc.vector.reciprocal(den[:], den[:])
                rec_r = osbp.tile([NH, 512], F32R, tag=f"recr{lc}",
                                  name=f"recr{lc}")
                nc.vector.tensor_copy(rec_r[:], den[:])
                for h in range(NH):
                    o_sb = stores[(h, lc)]
                    rr = nrm.tile([1, 512], F32R, tag="rr", name="rr")
                    nc.sync.dma_start(rr[:], rec_r[h:h + 1, :])
                    bc = pp.tile([128, 512], F32, tag="pp", name="bcn")
                    nc.tensor.matmul(bc[0:64, :], ones_r[:, 0:64], rr[:])
                    t, i = h // 4, (h % 4) // 2
                    sl = slice(lc * 1024 + i * 512, lc * 1024 + (i + 1) * 512)
                    if h % 2 == 0:
                        nc.vector.tensor_tensor(
                            xt8[t][0:64, sl], o_sb[0:64, :], bc[0:64, :],
                            AluOpType.mult)
                    else:
                        xb = nrm.tile([64, 512], F8, tag="xb", name="xb")
                        nc.vector.tensor_tensor(xb[:], o_sb[0:64, :],
                                                bc[0:64, :], AluOpType.mult)
                        nc.sync.dma_start(xt8[t][64:128, sl], xb[:])

            def proj_chunk(lc):
                sl = slice(lc * 512, (lc + 1) * 512)
                for it in range(NC4):
                    ps = pp.tile([128, 512], F32, tag="pp", name="psy")
                    for g in range(2):
                        off = (it * 2 + g) * 256
                        nc.tensor.matmul(
                            ps[:],
                            wp8[:, off:off + 256].rearrange(
                                "p (i m) -> p i m", i=2),
                            xt8[g][:, lc * 1024:(lc + 1) * 1024].rearrange(
                                "p (i n) -> p i n", i=2),
                            start=(g == 0), stop=(g == 1), perf_mode=DR)
                    # xres = ps/512 + (pet + bp)
                    nc.vector.scalar_tensor_tensor(
                        xres[it][:, sl], ps[:], 1.0 / 512, pet32[it][:, sl],
                        AluOpType.mult, AluOpType.add)

            stats = {}

            def ln_stats_chunk(lc):
                sl = slice(lc * 512, (lc + 1) * 512)
                psum = pp.tile([128, 512], F32, tag="pp", name="psum_sum")
                for c in range(NC4):
                    nc.tensor.matmul(psum[0:1, :], ones_c[:], xres[c][:, sl],
                                     start=(c == 0), stop=(c == NC4 - 1))
                psq = pp.tile([128, 512], F32, tag="pp", name="psum_sq")
                for c in range(NC4):
                    xsq = tmp.tile([128, 512], F32R, tag="xsq", name="xsq")
                    nc.vector.tensor_tensor(xsq[:], xres[c][:, sl],
                                            xres[c][:, sl], AluOpType.mult)
                    nc.tensor.matmul(psq[0:1, :], ones_c[:], xsq[:],
                                     start=(c == 0), stop=(c == NC4 - 1))
                mu = lrows.tile([1, 512], F32R, tag=f"mu{lc}", name=f"mu{lc}")
                ve = lrows.tile([1, 512], F32, tag="ve", name=f"ve{lc}")
                t0 = lrows.tile([1, 512], F32, tag="t0", name=f"t0{lc}")
                rstd = lrows.tile([1, 512], F32R, tag=f"rs{lc}", name=f"rs{lc}")
                nc.vector.tensor_scalar(mu[:], psum[0:1, :], 1.0 / C, None,
                                        AluOpType.mult)
                nc.vector.tensor_tensor(t0[:], mu[:], mu[:], AluOpType.mult)
                nc.vector.scalar_tensor_tensor(ve[:], psq[0:1, :], 1.0 / C,
                                               t0[:], AluOpType.mult,
                                               AluOpType.subtract)
                nc.vector.tensor_scalar(ve[:], ve[:], LN_EPS, None,
                                        AluOpType.add)
                nc.scalar.activation(t0[:], ve[:],
                                     mybir.ActivationFunctionType.Sqrt)
                r0 = lrows.tile([1, 512], F32, tag="r0", name=f"r0{lc}")
                nc.vector.reciprocal(r0[:], t0[:])
                nc.vector.tensor_tensor(t0[:], r0[:], r0[:], AluOpType.mult)
                nc.vector.tensor_tensor(t0[:], t0[:], ve[:], AluOpType.mult)
                nc.vector.tensor_scalar(t0[:], t0[:], -0.5, 1.5,
                                        AluOpType.mult, AluOpType.add)
                nc.vector.tensor_tensor(rstd[:], r0[:], t0[:], AluOpType.mult)
                stats[lc] = (mu, rstd)

            def ln_apply_chunk(lc):
                sl = slice(lc * 512, (lc + 1) * 512)
                mu, rstd = stats[lc]
                bmu = pp.tile([128, 512], F32, tag="pp", name="bmu")
                brs = pp.tile([128, 512], F32, tag="pp", name="brs")
                nc.tensor.matmul(bmu[:], ones_r[:], mu[:])
                nc.tensor.matmul(brs[:], ones_r[:], rstd[:])
                for c in range(NC4):
                    t = tmp.tile([128, 512], F32, tag="lnt", bufs=1,
                                 name="lnt")
                    y = yout.tile([128, 512], F32, tag="y", name="yout")
                    nc.vector.tensor_tensor(t[:], xres[c][:, sl], bmu[:],
                                            AluOpType.subtract)
                    nc.vector.tensor_tensor(t[:], t[:], brs[:],
                                            AluOpType.mult)
                    nc.vector.tensor_scalar(y[:], t[:], gamma[c][:],
                                            beta[c][:], AluOpType.mult,
                                            AluOpType.add)
                    nc.sync.dma_start(out_d[c * 128:(c + 1) * 128, sl], y[:])

            chunks = []
            if only_first:
                attention(0, 0)
            else:
                for lc in range(2):
                    for h in range(NH):
                        attention(h, lc)
                        if chunks:
                            chunks.pop(0)()
                    chunks += [lambda lc=lc: norm_chunk(lc),
                               lambda lc=lc: proj_chunk(lc),
                               lambda lc=lc: ln_stats_chunk(lc),
                               lambda lc=lc: ln_apply_chunk(lc)]
                while chunks:
                    chunks.pop(0)()

        if debug_taps and not only_first:
            for t in range(4):
                nc.sync.dma_start(taps["qt"][:, t * 2 * LQ:(t + 1) * 2 * LQ],
                                  qt8[t][:])
                nc.sync.dma_start(taps["kt"][:, t * 2 * L:(t + 1) * 2 * L],
                                  kt8[t][:])
            for t in range(2):
                nc.sync.dma_start(taps["xt"][:, t * 2 * LQ:(t + 1) * 2 * LQ],
                                  xt8[t][:])
            for mp in range(NMP):
                nc.sync.dma_start(
                    taps["vt"][:, mp * NH * 256:(mp + 1) * NH * 256],
                    vt8[mp][:])
            for i in range(NC4):
                nc.sync.dma_start(taps["xres"][i * 128:(i + 1) * 128, :],
                                  xres[i][:].bitcast(F32))
            for idx, ((h, lc), o_sb) in enumerate(sorted(stores.items())):
                nc.sync.dma_start(
                    taps["osb"][:, idx * 512:(idx + 1) * 512], o_sb[:])
        if debug_taps and only_first:
            nc.sync.dma_start(
                taps["osb"][:, 0:512], stores[(0, 0)][:])

    nc.compile()
    return nc


def prep_core_inputs(inputs):
    """Shard + lay out the full inputs for the 8 cores."""
    pet = np.asarray(inputs["pet_feat"], np.float32).reshape(B, C, L)
    ct = np.asarray(inputs["ct_feat"], np.float32).reshape(B, C, L)
    Wq = np.asarray(inputs["Wq"], np.float32)
    Wk = np.asarray(inputs["Wk"], np.float32)
    Wv = np.asarray(inputs["Wv"], np.float32)
    Wp = np.asarray(inputs["Wp"], np.float32)
    gamma = np.asarray(inputs["gamma"], np.float32).reshape(C, 1)
    beta = np.asarray(inputs["beta"], np.float32).reshape(C, 1)
    bp = np.asarray(inputs["bp"], np.float32).reshape(C, 1)

    def wqk8(Wx):
        # wq8[p, (t*2+g)*256 + ig*128 + (j*64+d)]
        #    = 16*Wx[64*(2t+j) + d, 256g + 128ig + p]
        A = (16 * Wx).reshape(4, 2, 64, 2, 2, 128)    # [t j d g ig p]
        return np.ascontiguousarray(
            A.transpose(5, 0, 3, 4, 1, 2).reshape(128, 2048)).astype(E4)

    wq8 = wqk8(Wq)
    wk8 = wqk8(Wk)
    # wv8[p, g*1024 + ig*512 + oc] = 16*Wv[oc, 256g+128ig+p]
    Bv = (16 * Wv).reshape(512, 2, 2, 128)            # [oc g ig p]
    wv8 = np.ascontiguousarray(
        Bv.transpose(3, 1, 2, 0).reshape(128, 2048)).astype(E4)
    # wp8[p, (it*2+g)*256 + ig*128 + m] = 16*Wp[128it+m, 256g+128ig+p]
    Cp = (16 * Wp).reshape(4, 128, 2, 2, 128)          # [it m g ig p]
    wp8 = np.ascontiguousarray(
        Cp.transpose(4, 0, 2, 3, 1).reshape(128, 2048)).astype(E4)

    def chan_pack(x, win):
        # [512, N] -> [128, 4N]; free = g*(2N) + c*(2*win) + ig*win + t
        N = x.shape[1]
        A = x.reshape(2, 2, 128, N // win, win)       # [g, ig, p, c, t]
        return np.ascontiguousarray(
            A.transpose(2, 0, 3, 1, 4).reshape(128, 4 * N)).astype(E4)

    vpad = np.zeros((128, NH, 2, 128), np.float32)
    vpad[:, :, :, HD] = 1.0
    vpad = vpad.astype(E4).reshape(128, NH * 256)

    qbias = np.zeros((128, LQ), E4)
    qbias[63, :] = E4(BIAS_Q)
    qbias[127, :] = E4(BIAS_Q)
    kbias = np.zeros((128, L), E4)
    kbias[63, :] = E4(BIAS_K)
    kbias[127, :] = E4(BIAS_K)

    in_maps = []
    for core in range(NCORES):
        b, j = divmod(core, 4)
        sl = slice(j * LQ, (j + 1) * LQ)
        pet_sl = np.ascontiguousarray(pet[b][:, sl])
        in_maps.append({
            "pet8": chan_pack(pet_sl, 512),
            "ct8k": chan_pack(ct[b], 512),
            "ct8v": chan_pack(ct[b], 128),
            "wq8": wq8, "wk8": wk8, "wv8": wv8, "wp8": wp8,
            "qbias8": qbias, "kbias8": kbias, "vpad8": vpad,
            "pet32p": pet_sl + bp,
            "gamma": gamma, "beta": beta,
        })
    return in_maps


def assemble_output(results):
    out = np.empty((B, C, L), np.float32)
    for core in range(NCORES):
        b, j = divmod(core, 4)
        out[b][:, j * LQ:(j + 1) * LQ] = results[core]["out"]
    return out.reshape(B, C, H, W)


_NC_CACHE = {}


def get_nc(debug=False):
    key = debug
    if key not in _NC_CACHE:
        _NC_CACHE[key] = build_nc(debug=debug)
    return _NC_CACHE[key]


def kernel(**inputs):
    nc = get_nc()
    in_maps = prep_core_inputs(inputs)
    res = bass_utils.run_bass_kernel_spmd(nc, in_maps, list(range(NCORES)))
    return assemble_output(res.results)
